# revision 24
# baseline (speedup 1.0000x reference)
"""Trainium2 Bass kernel for nn_AttentionLayer_10995116278518.

Computes softmax(einsum('sbe,e->bs', embedded, attn[:300])
              + einsum('sbf,f->bs', lstm_outputs, attn[300:]), axis=1)
(the reference's mask is computed-but-discarded, so it is unused here).

Sharding: data-parallel over batch. Each of the 8 cores handles 8 of the
64 batch rows; no cross-device communication.

The kernel is HBM-bandwidth bound (~36 MB/core at fp16), so everything
is built around clean DMA streaming:
  - host concatenates embedded+lstm features, casts to fp16 (validated
    against the 2e-2 tolerance; bf16 is NOT accurate enough), and lays
    the shard out feature-major: XT [4396 feats, 8b x 512s].
  - each 128-feature chunk is one contiguous ~1 MB DMA, alternating
    between the two HWDGE rings (sync / scalar engines, which do no
    other work during the stream).
  - TensorE does the dots: per chunk, 8 matmuls (one per batch row)
    with lhsT = attn-chunk replicated to 8 columns, rhs = that row's
    [128, 512] slice, accumulating into 8 PSUM banks out[8, 512].
    All 8 output rows of bank b are identical (= row b's logits), so
    row b is copied out same-partition — no transposes anywhere.
  - VectorE/ScalarE only do the final softmax (free-axis).
"""

import sys

import numpy as np

try:
    import concourse.bass as bass
except ImportError:  # stand-alone grading dir: the runtime lives here
    sys.path.insert(0, "/opt/trn_rl_repo")
    import concourse.bass as bass

import concourse.bacc as bacc
import concourse.tile as tile
from concourse import mybir
from concourse.bass_utils import run_bass_kernel_spmd

SEQ = 512
BATCH = 64
EMB = 300
LSTM = 4096
D = EMB + LSTM  # 4396
N_CORES = 8
BLOC = BATCH // N_CORES  # 8 batch rows per core
P = 128
R = BLOC * SEQ  # 4096 rows (b-major) per core
NCH = (D + P - 1) // P  # 35 feature chunks: 34 full + 1 of 44
KLAST = D - (NCH - 1) * P  # 44

F32 = mybir.dt.float32
F16 = mybir.dt.float16

PREFETCH = 18


def _build() -> bass.Bass:
    nc = bacc.Bacc()
    # feature-major fp16 shard: row f, column b*512+s
    x = nc.declare_dram_parameter("x", [D, R], F16, isOutput=False)
    # attn chunk c replicated to 8 columns: attn_rep[k, c, m] = attn[128c+k];
    # chunk NCH is all-zero (weights for the HAM keep-warm dummy matmuls)
    attn_rep = nc.declare_dram_parameter(
        "attn_rep", [P, NCH + 1, BLOC], F16, isOutput=False
    )
    out = nc.declare_dram_parameter("out", [BLOC, SEQ], F16, isOutput=True)

    with tile.TileContext(nc) as tc:
        with (
            tc.tile_pool(name="singles", bufs=1) as singles,
            tc.tile_pool(name="xtiles", bufs=20) as xpool,
            tc.tile_pool(name="psum", bufs=8, space="PSUM") as psum_pool,
        ):
            sb_attn = singles.tile([P, NCH + 1, BLOC], F16)
            nc.scalar.dma_start(out=sb_attn, in_=attn_rep[:, :, :])
            logits = singles.tile([BLOC, SEQ], F32)

            psums = []
            for b in range(BLOC):
                ps = psum_pool.tile([BLOC, SEQ], F32, tag="ps")
                psums.append(ps)

            tiles = {}
            def issue(c):
                kp = P if c < NCH - 1 else KLAST
                xt = xpool.tile([P, R], F16, tag="x")
                eng = nc.sync if c % 2 == 0 else nc.scalar
                if c == NCH - 2:
                    # split the last full chunk so its first matmuls can
                    # start half a transfer earlier (shorter PE drain)
                    h = R // 2
                    eng.dma_start(out=xt[0:kp, 0:h], in_=x[c * P : c * P + kp, 0:h])
                    eng.dma_start(out=xt[0:kp, h:R], in_=x[c * P : c * P + kp, h:R])
                else:
                    eng.dma_start(out=xt[0:kp, :], in_=x[c * P : c * P + kp, :])
                tiles[c] = xt

            # process the small partial chunk (34) BEFORE the last full
            # chunk (33): PSUM accumulation is commutative, and this hides
            # chunk 34's matmuls under chunk 33's DMA so the post-stream
            # PE drain is only chunk 33's matmuls
            proc = list(range(NCH - 2)) + [NCH - 1, NCH - 2]
            # issue order differs from proc order: chunk 34 is fetched
            # early (buffer slot 19 is never reused, so holding it until
            # its late matmuls blocks nothing) so that the final drain is
            # only chunk 33's matmuls
            iorder = (
                list(range(19)) + [NCH - 1] + list(range(19, NCH - 2)) + [NCH - 2]
            )
            for i in range(PREFETCH):
                issue(iorder[i])
            for i, c in enumerate(proc):
                if i + PREFETCH < NCH:
                    issue(iorder[i + PREFETCH])
                kp = P if c < NCH - 1 else KLAST
                xt = tiles.pop(c)
                for b in range(BLOC):
                    nc.tensor.matmul(
                        out=psums[b],
                        lhsT=sb_attn[0:kp, c, :],
                        rhs=xt[0:kp, b * SEQ : (b + 1) * SEQ],
                        start=(i == 0),
                        stop=(i == NCH - 1),
                        skip_group_check=True,
                    )
                # zero-weight dummy matmuls: add exactly 0 to a PSUM bank
                # but keep TensorE busy through DMA-wait gaps so the HAM
                # clock gate never re-throttles mid-stream (cold MMs slow
                # buffer recycling and stall the DMA stream). Skipped near
                # the end so they never delay the final drain.
                if i < NCH - 3:
                    for j in range(2):
                        nc.tensor.matmul(
                            out=psums[(2 * c + j) % BLOC],
                            lhsT=sb_attn[0:kp, NCH, :],
                            rhs=xt[0:kp, j * SEQ : (j + 1) * SEQ],
                            start=False,
                            stop=False,
                            skip_group_check=True,
                        )

            # bank b's rows are all identical (= logits for batch row b).
            # Engines can't write partition b directly (quadrant alignment),
            # so stage row 0 of each bank into a flat partition-0 row, then
            # scatter segments to partitions 0..7 with tiny SBUF->SBUF DMAs.
            s0 = singles.tile([1, BLOC * SEQ], F32)
            for b in range(BLOC):
                seg = s0[0:1, b * SEQ : (b + 1) * SEQ]
                src = psums[b][0:1, :]
                if b % 2 == 1:
                    nc.scalar.copy(seg, src)
                else:
                    nc.vector.tensor_scalar_mul(seg, src, 1.0)
            # two half-scatters on separate rings (dma_start only requires
            # equal total element counts, not equal shapes): rows 0-3 fly
            # while the b4-7 copies still run, and the final gating
            # transfer is half the bytes
            H4 = 4 * SEQ
            nc.sync.dma_start(out=logits[0:4, :], in_=s0[0:1, 0:H4])
            # second half also on sync: the ACT engine's DMA trigger is
            # ~2x slower (~1.2us) and sync is idle here anyway
            nc.sync.dma_start(out=logits[4:8, :], in_=s0[0:1, H4 : 2 * H4])

            # softmax along s (free axis)
            nm = singles.tile([BLOC, 1], F32)
            ssum = singles.tile([BLOC, 1], F32)
            rec = singles.tile([BLOC, 1], F32)
            expt = singles.tile([BLOC, SEQ], F32)
            res = singles.tile([BLOC, SEQ], F16)
            nc.vector.tensor_reduce(
                out=nm,
                in_=logits,
                axis=mybir.AxisListType.X,
                op=mybir.AluOpType.max,
                negate=True,
            )
            nc.scalar.activation(
                out=expt,
                in_=logits,
                func=mybir.ActivationFunctionType.Exp,
                bias=nm,
                scale=1.0,
                accum_out=ssum,
            )
            nc.vector.reciprocal(rec, ssum)
            nc.vector.tensor_scalar_mul(res, expt, rec)
            # output split across both rings: parallel triggers and
            # parallel half-size DRAM write receipts
            nc.sync.dma_start(out=out[0:4, :], in_=res[0:4, :])
            nc.scalar.dma_start(out=out[4:8, :], in_=res[4:8, :])

    nc.compile()
    return nc


_NC_CACHE = None


def _get_nc() -> bass.Bass:
    global _NC_CACHE
    if _NC_CACHE is None:
        _NC_CACHE = _build()
    return _NC_CACHE


def _make_in_maps(embedded, lstm_outputs, attn):
    embedded = np.asarray(embedded, dtype=np.float32)
    lstm_outputs = np.asarray(lstm_outputs, dtype=np.float32)
    attn = np.asarray(attn, dtype=np.float32).astype(np.float16)
    # [S, B, F] -> [s, core, b, F]
    emb4 = embedded.reshape(SEQ, N_CORES, BLOC, EMB)
    lst4 = lstm_outputs.reshape(SEQ, N_CORES, BLOC, LSTM)
    att_rep = np.zeros((P, NCH + 1, BLOC), dtype=np.float16)
    for c in range(NCH):
        kp = P if c < NCH - 1 else KLAST
        att_rep[:kp, c, :] = attn[c * P : c * P + kp, None]
    in_maps = []
    for i in range(N_CORES):
        xs = np.empty((D, R), dtype=np.float16)
        # [s, b, F] -> [F, b, s] -> [F, b*512+s]
        xs[:EMB] = emb4[:, i].transpose(2, 1, 0).reshape(EMB, R)
        xs[EMB:] = lst4[:, i].transpose(2, 1, 0).reshape(LSTM, R)
        in_maps.append({"x": xs, "attn_rep": att_rep})
    return in_maps


def _run(embedded, lstm_outputs, attn, trace=False, **spmd_kwargs):
    nc = _get_nc()
    in_maps = _make_in_maps(embedded, lstm_outputs, attn)
    r = run_bass_kernel_spmd(
        nc, in_maps, core_ids=list(range(N_CORES)), trace=trace, **spmd_kwargs
    )
    out = np.concatenate([r.results[i]["out"] for i in range(N_CORES)], axis=0)
    return out, r


def kernel(embedded, lstm_outputs, attn, mask=None, **_ignored) -> np.ndarray:
    out, _ = _run(embedded, lstm_outputs, attn, trace=False)
    return out.astype(np.float32)


# revision 25
# speedup vs baseline: 1.0237x; 1.0237x over previous
"""Trainium2 Bass kernel for nn_AttentionLayer_10995116278518.

Computes softmax(einsum('sbe,e->bs', embedded, attn[:300])
              + einsum('sbf,f->bs', lstm_outputs, attn[300:]), axis=1)
(the reference's mask is computed-but-discarded, so it is unused here).

Sharding: data-parallel over batch. Each of the 8 cores handles 8 of the
64 batch rows; no cross-device communication.

The kernel is HBM-bandwidth bound (~36 MB/core at fp16), so everything
is built around clean DMA streaming:
  - host concatenates embedded+lstm features, casts to fp16 (validated
    against the 2e-2 tolerance; bf16 is NOT accurate enough), and lays
    the shard out feature-major: XT [4396 feats, 8b x 512s].
  - each 128-feature chunk is one contiguous ~1 MB DMA, alternating
    between the two HWDGE rings (sync / scalar engines, which do no
    other work during the stream).
  - TensorE does the dots: per chunk, 8 matmuls (one per batch row)
    with lhsT = attn-chunk replicated to 8 columns, rhs = that row's
    [128, 512] slice, accumulating into 8 PSUM banks out[8, 512].
    All 8 output rows of bank b are identical (= row b's logits), so
    row b is copied out same-partition — no transposes anywhere.
  - VectorE/ScalarE only do the final softmax (free-axis).
"""

import sys

import numpy as np

try:
    import concourse.bass as bass
except ImportError:  # stand-alone grading dir: the runtime lives here
    sys.path.insert(0, "/opt/trn_rl_repo")
    import concourse.bass as bass

import concourse.bacc as bacc
import concourse.tile as tile
from concourse import mybir
from concourse.bass_utils import run_bass_kernel_spmd

SEQ = 512
BATCH = 64
EMB = 300
LSTM = 4096
D = EMB + LSTM  # 4396
N_CORES = 8
BLOC = BATCH // N_CORES  # 8 batch rows per core
P = 128
R = BLOC * SEQ  # 4096 rows (b-major) per core
NCH = (D + P - 1) // P  # 35 feature chunks: 34 full + 1 of 44
KLAST = D - (NCH - 1) * P  # 44

F32 = mybir.dt.float32
F16 = mybir.dt.float16

PREFETCH = 18


def _build() -> bass.Bass:
    nc = bacc.Bacc()
    # feature-major fp16 shard: row f, column b*512+s
    x = nc.declare_dram_parameter("x", [D, R], F16, isOutput=False)
    # attn chunk c replicated to 8 columns: attn_rep[k, c, m] = attn[128c+k];
    # chunk NCH is all-zero (weights for the HAM keep-warm dummy matmuls)
    attn_rep = nc.declare_dram_parameter(
        "attn_rep", [P, NCH + 1, BLOC], F16, isOutput=False
    )
    out = nc.declare_dram_parameter("out", [BLOC, SEQ], F16, isOutput=True)

    with tile.TileContext(nc) as tc:
        with (
            tc.tile_pool(name="singles", bufs=1) as singles,
            tc.tile_pool(name="xtiles", bufs=20) as xpool,
            tc.tile_pool(name="psum", bufs=8, space="PSUM") as psum_pool,
        ):
            sb_attn = singles.tile([P, NCH + 1, BLOC], F16)
            nc.scalar.dma_start(out=sb_attn, in_=attn_rep[:, :, :])
            logits = singles.tile([BLOC, SEQ], F32)

            psums = []
            for b in range(BLOC):
                ps = psum_pool.tile([BLOC, SEQ], F32, tag="ps")
                psums.append(ps)

            tiles = {}
            def issue(c):
                kp = P if c < NCH - 1 else KLAST
                xt = xpool.tile([P, R], F16, tag="x")
                eng = nc.sync if c % 2 == 0 else nc.scalar
                if c == NCH - 2:
                    # split the last full chunk so its first matmuls can
                    # start half a transfer earlier (shorter PE drain)
                    h = R // 2
                    eng.dma_start(out=xt[0:kp, 0:h], in_=x[c * P : c * P + kp, 0:h])
                    eng.dma_start(out=xt[0:kp, h:R], in_=x[c * P : c * P + kp, h:R])
                else:
                    eng.dma_start(out=xt[0:kp, :], in_=x[c * P : c * P + kp, :])
                tiles[c] = xt

            # process the small partial chunk (34) BEFORE the last full
            # chunk (33): PSUM accumulation is commutative, and this hides
            # chunk 34's matmuls under chunk 33's DMA so the post-stream
            # PE drain is only chunk 33's matmuls
            proc = list(range(NCH - 2)) + [NCH - 1, NCH - 2]
            # issue order differs from proc order: chunk 34 is fetched
            # early (buffer slot 19 is never reused, so holding it until
            # its late matmuls blocks nothing) so that the final drain is
            # only chunk 33's matmuls
            iorder = (
                list(range(19)) + [NCH - 1] + list(range(19, NCH - 2)) + [NCH - 2]
            )
            for i in range(PREFETCH):
                issue(iorder[i])
            for i, c in enumerate(proc):
                if i + PREFETCH < NCH:
                    issue(iorder[i + PREFETCH])
                kp = P if c < NCH - 1 else KLAST
                xt = tiles.pop(c)
                for b in range(BLOC):
                    nc.tensor.matmul(
                        out=psums[b],
                        lhsT=sb_attn[0:kp, c, :],
                        rhs=xt[0:kp, b * SEQ : (b + 1) * SEQ],
                        start=(i == 0),
                        stop=(i == NCH - 1),
                        skip_group_check=True,
                    )
                # zero-weight dummy matmuls: add exactly 0 to a PSUM bank
                # but keep TensorE busy through DMA-wait gaps so the HAM
                # clock gate never re-throttles mid-stream (cold MMs slow
                # buffer recycling and stall the DMA stream). Skipped near
                # the end so they never delay the final drain.
                if i < NCH - 3:
                    for j in range(2):
                        nc.tensor.matmul(
                            out=psums[(2 * c + j) % BLOC],
                            lhsT=sb_attn[0:kp, NCH, :],
                            rhs=xt[0:kp, j * SEQ : (j + 1) * SEQ],
                            start=False,
                            stop=False,
                            skip_group_check=True,
                        )

            # bank b's rows are all identical (= logits for batch row b).
            # Engines can't write partition b directly (quadrant alignment),
            # so stage row 0 of each bank into a flat partition-0 row, then
            # scatter segments to partitions 0..7 with tiny SBUF->SBUF DMAs.
            s0 = singles.tile([1, BLOC * SEQ], F32)
            for b in range(BLOC):
                seg = s0[0:1, b * SEQ : (b + 1) * SEQ]
                src = psums[b][0:1, :]
                if b % 2 == 1:
                    nc.scalar.copy(seg, src)
                else:
                    nc.vector.tensor_scalar_mul(seg, src, 1.0)
            # two half-scatters on separate rings (dma_start only requires
            # equal total element counts, not equal shapes): rows 0-3 fly
            # while the b4-7 copies still run, and the final gating
            # transfer is half the bytes
            H4 = 4 * SEQ
            nc.sync.dma_start(out=logits[0:4, :], in_=s0[0:1, 0:H4])
            nc.scalar.dma_start(out=logits[4:8, :], in_=s0[0:1, H4 : 2 * H4])

            # softmax along s (free axis)
            nm = singles.tile([BLOC, 1], F32)
            ssum = singles.tile([BLOC, 1], F32)
            rec = singles.tile([BLOC, 1], F32)
            expt = singles.tile([BLOC, SEQ], F32)
            res = singles.tile([BLOC, SEQ], F16)
            nc.vector.tensor_reduce(
                out=nm,
                in_=logits,
                axis=mybir.AxisListType.X,
                op=mybir.AluOpType.max,
                negate=True,
            )
            nc.scalar.activation(
                out=expt,
                in_=logits,
                func=mybir.ActivationFunctionType.Exp,
                bias=nm,
                scale=1.0,
                accum_out=ssum,
            )
            nc.vector.reciprocal(rec, ssum)
            nc.vector.tensor_scalar_mul(res, expt, rec)
            nc.sync.dma_start(out=out[:, :], in_=res)

    nc.compile()
    return nc


_NC_CACHE = None


def _get_nc() -> bass.Bass:
    global _NC_CACHE
    if _NC_CACHE is None:
        _NC_CACHE = _build()
    return _NC_CACHE


def _make_in_maps(embedded, lstm_outputs, attn):
    embedded = np.asarray(embedded, dtype=np.float32)
    lstm_outputs = np.asarray(lstm_outputs, dtype=np.float32)
    attn = np.asarray(attn, dtype=np.float32).astype(np.float16)
    # [S, B, F] -> [s, core, b, F]
    emb4 = embedded.reshape(SEQ, N_CORES, BLOC, EMB)
    lst4 = lstm_outputs.reshape(SEQ, N_CORES, BLOC, LSTM)
    att_rep = np.zeros((P, NCH + 1, BLOC), dtype=np.float16)
    for c in range(NCH):
        kp = P if c < NCH - 1 else KLAST
        att_rep[:kp, c, :] = attn[c * P : c * P + kp, None]
    in_maps = []
    for i in range(N_CORES):
        xs = np.empty((D, R), dtype=np.float16)
        # [s, b, F] -> [F, b, s] -> [F, b*512+s]
        xs[:EMB] = emb4[:, i].transpose(2, 1, 0).reshape(EMB, R)
        xs[EMB:] = lst4[:, i].transpose(2, 1, 0).reshape(LSTM, R)
        in_maps.append({"x": xs, "attn_rep": att_rep})
    return in_maps


def _run(embedded, lstm_outputs, attn, trace=False, **spmd_kwargs):
    nc = _get_nc()
    in_maps = _make_in_maps(embedded, lstm_outputs, attn)
    r = run_bass_kernel_spmd(
        nc, in_maps, core_ids=list(range(N_CORES)), trace=trace, **spmd_kwargs
    )
    out = np.concatenate([r.results[i]["out"] for i in range(N_CORES)], axis=0)
    return out, r


def kernel(embedded, lstm_outputs, attn, mask=None, **_ignored) -> np.ndarray:
    out, _ = _run(embedded, lstm_outputs, attn, trace=False)
    return out.astype(np.float32)
